# revision 9
# baseline (speedup 1.0000x reference)
"""Trainium2 Bass kernel for additive (Bahdanau) attention scores.

reference:
    pq = q @ Wq.T            (B, Q, H)
    pk = k @ Wk.T            (B, K, H)
    scores[b,q,k] = sum_h w[h] * tanh(pq[b,q,h] + pk[b,k,h])
    scores[b,q,k>=S[b]] = -inf

Shapes: B=4, Q=K=512, D=512, H=256, f32.

Strategy (8 NeuronCores, SPMD):
  - Core c owns q-rows [c*64, (c+1)*64) of ALL four batches; k is replicated.
    Since every core sees every batch, per-batch compile-time constants
    (the valid-k prefix N_b derived from S) keep the program identical
    across cores -> clean SPMD with perfect load balance.
  - On device, h lives on partitions (H=256 -> two 128-chunks).  Per
    (b, q-row, h-chunk) a single ScalarE activation computes
    tanh(PK_T + bias) with bias = PQ_T[:, q] (fused add+tanh), then a
    bf16 M=1 TensorE matmul against w reduces over h into a PSUM row,
    accumulating both h-chunks; the row DMAs straight to DRAM.
  - Keys k >= S[b] only ever reach -inf in the output, so the kernel
    computes just the valid prefix (rounded up); the host writes the
    -inf region.
"""

import sys

sys.path.insert(0, "/opt/trn_rl_repo")

import numpy as np

import concourse.bass as bass  # noqa: E402
import concourse.mybir as mybir  # noqa: E402
from concourse import bacc  # noqa: E402
from concourse.bass_utils import run_bass_kernel_spmd  # noqa: E402
from concourse.masks import make_identity  # noqa: E402
from concourse.tile import TileContext  # noqa: E402

B, Q, K, D, H = 4, 512, 512, 512, 256
NCORES = 8
QSH = Q // NCORES  # q rows per core per batch
F32 = mybir.dt.float32
BF16 = mybir.dt.bfloat16
TANH = mybir.ActivationFunctionType.Tanh

_CACHE = {}


def build(nb):
    """Build the per-core Bass program. nb = per-batch valid-k extents."""
    nc = bacc.Bacc(trn_type="TRN2")

    q_ext = nc.declare_dram_parameter("q", [B, QSH, D], F32, isOutput=False)
    k_ext = nc.declare_dram_parameter("k", [B, K, D], F32, isOutput=False)
    wqT_ext = nc.declare_dram_parameter("wqT", [D, H], F32, isOutput=False)
    wkT_ext = nc.declare_dram_parameter("wkT", [D, H], F32, isOutput=False)
    w_ext = nc.declare_dram_parameter("w", [2, 128, 1], F32, isOutput=False)
    out_ext = nc.declare_dram_parameter("out", [B, QSH, K], F32, isOutput=True)

    with TileContext(nc) as tc:
        with (
            tc.tile_pool(name="const", bufs=1) as const,
            tc.tile_pool(name="persist", bufs=1) as persist,
            tc.tile_pool(name="stage", bufs=3) as stage,
            tc.tile_pool(name="kt", bufs=2) as ktp,
            tc.tile_pool(name="tp", bufs=4) as tp,
            tc.tile_pool(name="tpsum", bufs=2, space="PSUM") as tpsum,
            tc.tile_pool(name="ppsum", bufs=2, space="PSUM") as ppsum,
            tc.tile_pool(name="spsum", bufs=4, space="PSUM") as spsum,
        ):
            ident = const.tile([128, 128], F32)
            make_identity(nc, ident)

            # weights: WqT/WkT as (d-chunk, 128d, H) in SBUF
            wqT_sb = const.tile([128, 4, H], F32)
            wkT_sb = const.tile([128, 4, H], F32)
            for dc in range(4):
                nc.sync.dma_start(
                    out=wqT_sb[:, dc, :], in_=wqT_ext[dc * 128 : (dc + 1) * 128, :]
                )
                nc.sync.dma_start(
                    out=wkT_sb[:, dc, :], in_=wkT_ext[dc * 128 : (dc + 1) * 128, :]
                )
            w_f32 = const.tile([128, 2], F32)
            for hc in range(2):
                nc.sync.dma_start(out=w_f32[:, hc : hc + 1], in_=w_ext[hc])
            w_bf = const.tile([128, 2], BF16)
            nc.vector.tensor_copy(w_bf[:], w_f32[:])

            # persistent projections
            pq_all = persist.tile([128, B, 2, QSH], F32, tag="pqa")
            pk_all = persist.tile([128, B, 2, K], F32, tag="pka")

            for b in range(B):
                n = nb[b]
                # ---- PQ_T[b] : (2*128h, QSH) ----
                q_sb = stage.tile([QSH, D], F32, tag="qsb")
                nc.sync.dma_start(out=q_sb[:], in_=q_ext[b])
                qT_sb = stage.tile([128, 4, QSH], F32, tag="qT")
                for dc in range(4):
                    tps = tpsum.tile([128, 128], F32, tag="tr")
                    nc.tensor.transpose(
                        tps[:, :QSH], q_sb[:, dc * 128 : (dc + 1) * 128], ident[:QSH, :QSH]
                    )
                    nc.vector.tensor_copy(qT_sb[:, dc, :], tps[:, :QSH])
                for hc in range(2):
                    pps = ppsum.tile([128, 512], F32, tag="proj")
                    for dc in range(4):
                        nc.tensor.matmul(
                            pps[:, :QSH],
                            wqT_sb[:, dc, hc * 128 : (hc + 1) * 128],
                            qT_sb[:, dc, :],
                            start=(dc == 0),
                            stop=(dc == 3),
                        )
                    nc.vector.tensor_copy(pq_all[:, b, hc, :], pps[:, :QSH])

                # ---- PK_T[b] : (2*128h, n) ----
                nkc = (n + 127) // 128
                kT_sb = ktp.tile([128, 4, 512], F32, tag="kT")
                for kc in range(nkc):
                    rows = min(128, n - kc * 128)
                    k_sb = stage.tile([128, D], F32, tag="ksb")
                    nc.sync.dma_start(
                        out=k_sb[:rows, :], in_=k_ext[b, kc * 128 : kc * 128 + rows, :]
                    )
                    for dc in range(4):
                        tps = tpsum.tile([128, 128], F32, tag="tr")
                        nc.tensor.transpose(
                            tps[:, :rows],
                            k_sb[:rows, dc * 128 : (dc + 1) * 128],
                            ident[:rows, :rows],
                        )
                        nc.vector.tensor_copy(
                            kT_sb[:, dc, kc * 128 : kc * 128 + rows], tps[:, :rows]
                        )
                for hc in range(2):
                    pps = ppsum.tile([128, 512], F32, tag="proj")
                    for dc in range(4):
                        nc.tensor.matmul(
                            pps[:, :n],
                            wkT_sb[:, dc, hc * 128 : (hc + 1) * 128],
                            kT_sb[:, dc, :n],
                            start=(dc == 0),
                            stop=(dc == 3),
                        )
                    nc.vector.tensor_copy(pk_all[:, b, hc, :n], pps[:, :n])

            # ---- hot loop ----
            # Scores accumulate transposed: per (b, k-tile) a PSUM tile
            # (k on partitions, q on free).  Each (q, h-chunk) matmul uses
            # T as the stationary operand and w as the single moving
            # column, writing score column q.  The dense tile then gets
            # copied to SBUF, PE-transposed back to (q, k), and DMA'd out.
            for b in range(B):
                n = nb[b]
                nkc = (n + 127) // 128
                sc_tiles = []
                for kc in range(nkc):
                    sc_tiles.append(
                        spsum.tile([128, QSH], F32, tag="sc", name=f"sc_{b}_{kc}")
                    )
                for qj in range(QSH):
                    for hc in range(2):
                        t_bf = tp.tile([128, 512], BF16, tag="t")
                        nc.scalar.activation(
                            t_bf[:, :n],
                            pk_all[:, b, hc, :n],
                            TANH,
                            bias=pq_all[:, b, hc, qj : qj + 1],
                        )
                        for kc in range(nkc):
                            rows = min(128, n - kc * 128)
                            nc.tensor.matmul(
                                sc_tiles[kc][:rows, qj : qj + 1],
                                t_bf[:, kc * 128 : kc * 128 + rows],
                                w_bf[:, hc : hc + 1],
                                start=(hc == 0),
                                stop=(hc == 1),
                            )
                for kc in range(nkc):
                    rows = min(128, n - kc * 128)
                    sc_sb = stage.tile([128, QSH], F32, tag="scsb")
                    nc.vector.tensor_copy(sc_sb[:rows, :], sc_tiles[kc][:rows, :])
                    so_ps = tpsum.tile([128, 128], F32, tag="tr")
                    nc.tensor.transpose(
                        so_ps[:QSH, :rows], sc_sb[:rows, :], ident[:rows, :rows]
                    )
                    so_sb = stage.tile([QSH, 128], F32, tag="sosb")
                    nc.vector.tensor_copy(so_sb[:, :rows], so_ps[:QSH, :rows])
                    nc.sync.dma_start(
                        out=out_ext[b, :, kc * 128 : kc * 128 + rows],
                        in_=so_sb[:, :rows],
                    )
    nc.finalize()
    return nc


def _round_up(x, m):
    return ((x + m - 1) // m) * m


def kernel(q, k, v, S, Wq, Wk, w):
    q = np.asarray(q, dtype=np.float32)
    k = np.asarray(k, dtype=np.float32)
    S_np = np.asarray(S).astype(np.int64)
    Wq = np.asarray(Wq, dtype=np.float32)
    Wk = np.asarray(Wk, dtype=np.float32)
    w = np.asarray(w, dtype=np.float32)

    nb = tuple(int(min(K, max(32, _round_up(int(s), 32)))) for s in S_np)

    if nb not in _CACHE:
        _CACHE[nb] = build(nb)
    nc = _CACHE[nb]

    wqT = np.ascontiguousarray(Wq.T)  # (D, H)
    wkT = np.ascontiguousarray(Wk.T)  # (D, H)
    w_r = np.ascontiguousarray(w.reshape(2, 128, 1))

    in_maps = []
    for c in range(NCORES):
        in_maps.append(
            {
                "q": np.ascontiguousarray(q[:, c * QSH : (c + 1) * QSH, :]),
                "k": k,
                "wqT": wqT,
                "wkT": wkT,
                "w": w_r,
            }
        )

    res = run_bass_kernel_spmd(nc, in_maps, core_ids=list(range(NCORES)))
    outs = res.results

    full = np.empty((B, Q, K), dtype=np.float32)
    for c in range(NCORES):
        full[:, c * QSH : (c + 1) * QSH, :] = outs[c]["out"]
    for b in range(B):
        full[b, :, int(S_np[b]) :] = -np.inf
    return full


# revision 12
# speedup vs baseline: 1.4417x; 1.4417x over previous
"""Trainium2 Bass kernel for additive (Bahdanau) attention scores.

reference:
    pq = q @ Wq.T            (B, Q, H)
    pk = k @ Wk.T            (B, K, H)
    scores[b,q,k] = sum_h w[h] * tanh(pq[b,q,h] + pk[b,k,h])
    scores[b,q,k>=S[b]] = -inf

Shapes: B=4, Q=K=512, D=512, H=256, f32.

Strategy (8 NeuronCores, SPMD):
  - Core c owns q-rows [c*64, (c+1)*64) of ALL four batches; k is replicated.
    Since every core sees every batch, per-batch compile-time constants
    (the valid-k prefix N_b derived from S) keep the program identical
    across cores -> clean SPMD with perfect load balance.
  - On device, h lives on partitions (H=256 -> two 128-chunks).  Per
    (b, q-row, h-chunk) a single ScalarE activation computes
    tanh(PK_T + bias) with bias = PQ_T[:, q] (fused add+tanh), then a
    bf16 M=1 TensorE matmul against w reduces over h into a PSUM row,
    accumulating both h-chunks; the row DMAs straight to DRAM.
  - Keys k >= S[b] only ever reach -inf in the output, so the kernel
    computes just the valid prefix (rounded up); the host writes the
    -inf region.
"""

import sys

sys.path.insert(0, "/opt/trn_rl_repo")

import numpy as np

import concourse.bass as bass  # noqa: E402
import concourse.mybir as mybir  # noqa: E402
from concourse import bacc  # noqa: E402
from concourse.bass_utils import run_bass_kernel_spmd  # noqa: E402
from concourse.masks import make_identity  # noqa: E402
from concourse.tile import TileContext  # noqa: E402

B, Q, K, D, H = 4, 512, 512, 512, 256
NCORES = 8
QSH = Q // NCORES  # q rows per core per batch
F32 = mybir.dt.float32
BF16 = mybir.dt.bfloat16
TANH = mybir.ActivationFunctionType.Tanh

_CACHE = {}


def build(nb):
    """Build the per-core Bass program. nb = per-batch valid-k extents."""
    nc = bacc.Bacc(trn_type="TRN2")

    q_ext = nc.declare_dram_parameter("q", [B, QSH, D], F32, isOutput=False)
    k_ext = nc.declare_dram_parameter("k", [B, K, D], F32, isOutput=False)
    wqT_ext = nc.declare_dram_parameter("wqT", [D, H], F32, isOutput=False)
    wkT_ext = nc.declare_dram_parameter("wkT", [D, H], F32, isOutput=False)
    w_ext = nc.declare_dram_parameter("w", [2, 128, 1], F32, isOutput=False)
    out_ext = nc.declare_dram_parameter("out", [B, QSH, K], F32, isOutput=True)

    with TileContext(nc) as tc:
        with (
            tc.tile_pool(name="const", bufs=1) as const,
            tc.tile_pool(name="persist", bufs=1) as persist,
            tc.tile_pool(name="stage", bufs=3) as stage,
            tc.tile_pool(name="kt", bufs=2) as ktp,
            tc.tile_pool(name="tp", bufs=4) as tp,
            tc.tile_pool(name="tpsum", bufs=2, space="PSUM") as tpsum,
            tc.tile_pool(name="ppsum", bufs=2, space="PSUM") as ppsum,
            tc.tile_pool(name="spsum", bufs=4, space="PSUM") as spsum,
        ):
            ident = const.tile([128, 128], F32)
            make_identity(nc, ident)

            # weights: WqT/WkT as (d-chunk, 128d, H) in SBUF
            wqT_sb = const.tile([128, 4, H], F32)
            wkT_sb = const.tile([128, 4, H], F32)
            for dc in range(4):
                nc.sync.dma_start(
                    out=wqT_sb[:, dc, :], in_=wqT_ext[dc * 128 : (dc + 1) * 128, :]
                )
                nc.sync.dma_start(
                    out=wkT_sb[:, dc, :], in_=wkT_ext[dc * 128 : (dc + 1) * 128, :]
                )
            w_f32 = const.tile([128, 2], F32)
            for hc in range(2):
                nc.sync.dma_start(out=w_f32[:, hc : hc + 1], in_=w_ext[hc])
            w_bf = const.tile([128, 2], BF16)
            nc.vector.tensor_copy(w_bf[:], w_f32[:])

            # persistent projections (pk in bf16: feeds the DVE 4x-mode adds)
            pq_all = persist.tile([128, B, 2, QSH], F32, tag="pqa")
            pk_bf = persist.tile([128, B, 2, K], BF16, tag="pka")

            for b in range(B):
                n = nb[b]
                # ---- PQ_T[b] : (2*128h, QSH) ----
                q_sb = stage.tile([QSH, D], F32, tag="qsb")
                nc.sync.dma_start(out=q_sb[:], in_=q_ext[b])
                qT_sb = stage.tile([128, 4, QSH], F32, tag="qT")
                for dc in range(4):
                    tps = tpsum.tile([128, 128], F32, tag="tr")
                    nc.tensor.transpose(
                        tps[:, :QSH], q_sb[:, dc * 128 : (dc + 1) * 128], ident[:QSH, :QSH]
                    )
                    nc.vector.tensor_copy(qT_sb[:, dc, :], tps[:, :QSH])
                for hc in range(2):
                    pps = ppsum.tile([128, 512], F32, tag="proj")
                    for dc in range(4):
                        nc.tensor.matmul(
                            pps[:, :QSH],
                            wqT_sb[:, dc, hc * 128 : (hc + 1) * 128],
                            qT_sb[:, dc, :],
                            start=(dc == 0),
                            stop=(dc == 3),
                        )
                    nc.vector.tensor_copy(pq_all[:, b, hc, :], pps[:, :QSH])

                # ---- PK_T[b] : (2*128h, n) ----
                nkc = (n + 127) // 128
                kT_sb = ktp.tile([128, 4, 512], F32, tag="kT")
                for kc in range(nkc):
                    rows = min(128, n - kc * 128)
                    k_sb = stage.tile([128, D], F32, tag="ksb")
                    nc.sync.dma_start(
                        out=k_sb[:rows, :], in_=k_ext[b, kc * 128 : kc * 128 + rows, :]
                    )
                    for dc in range(4):
                        tps = tpsum.tile([128, 128], F32, tag="tr")
                        nc.tensor.transpose(
                            tps[:, :rows],
                            k_sb[:rows, dc * 128 : (dc + 1) * 128],
                            ident[:rows, :rows],
                        )
                        nc.vector.tensor_copy(
                            kT_sb[:, dc, kc * 128 : kc * 128 + rows], tps[:, :rows]
                        )
                for hc in range(2):
                    pps = ppsum.tile([128, 512], F32, tag="proj")
                    for dc in range(4):
                        nc.tensor.matmul(
                            pps[:, :n],
                            wkT_sb[:, dc, hc * 128 : (hc + 1) * 128],
                            kT_sb[:, dc, :n],
                            start=(dc == 0),
                            stop=(dc == 3),
                        )
                    nc.vector.tensor_copy(pk_bf[:, b, hc, :n], pps[:, :n])

            # ---- hot loop ----
            # Scores accumulate transposed: per (b, k-tile) a PSUM tile
            # (k on partitions, q on free).  Each (q, h-chunk) matmul uses
            # T as the stationary operand and w as the single moving
            # column, writing score column q.  The dense tile then gets
            # copied to SBUF, PE-transposed back to (q, k), and DMA'd out.
            for b in range(B):
                n = nb[b]
                nkc = (n + 127) // 128
                sc_tiles = []
                for kc in range(nkc):
                    sc_tiles.append(
                        spsum.tile([128, QSH], F32, tag="sc", name=f"sc_{b}_{kc}")
                    )
                G = 8  # q rows per activation stack
                for g in range(QSH // G):
                    t_stacks = []
                    for hc in range(2):
                        x_st = tp.tile(
                            [128, G * 512], BF16, tag=f"x{hc}", name=f"x_{b}_{g}_{hc}"
                        )
                        for j in range(G):
                            qj = g * G + j
                            nc.vector.tensor_scalar_add(
                                x_st[:, j * n : j * n + n],
                                pk_bf[:, b, hc, :n],
                                pq_all[:, b, hc, qj : qj + 1],
                            )
                        t_st = tp.tile(
                            [128, G * 512], BF16, tag=f"t{hc}", name=f"t_{b}_{g}_{hc}"
                        )
                        nc.scalar.activation(
                            t_st[:, : G * n], x_st[:, : G * n], TANH
                        )
                        t_stacks.append(t_st)
                    for j in range(G):
                        qj = g * G + j
                        for kc in range(nkc):
                            rows = min(128, n - kc * 128)
                            for hc in range(2):
                                nc.tensor.matmul(
                                    sc_tiles[kc][:rows, qj : qj + 1],
                                    t_stacks[hc][
                                        :, j * n + kc * 128 : j * n + kc * 128 + rows
                                    ],
                                    w_bf[:, hc : hc + 1],
                                    start=(hc == 0),
                                    stop=(hc == 1),
                                )
                for kc in range(nkc):
                    rows = min(128, n - kc * 128)
                    sc_sb = stage.tile([128, QSH], F32, tag="scsb")
                    nc.vector.tensor_copy(sc_sb[:rows, :], sc_tiles[kc][:rows, :])
                    so_ps = tpsum.tile([128, 128], F32, tag="tr")
                    nc.tensor.transpose(
                        so_ps[:QSH, :rows], sc_sb[:rows, :], ident[:rows, :rows]
                    )
                    so_sb = stage.tile([QSH, 128], F32, tag="sosb")
                    nc.vector.tensor_copy(so_sb[:, :rows], so_ps[:QSH, :rows])
                    nc.sync.dma_start(
                        out=out_ext[b, :, kc * 128 : kc * 128 + rows],
                        in_=so_sb[:, :rows],
                    )
    nc.finalize()
    return nc


def _round_up(x, m):
    return ((x + m - 1) // m) * m


def kernel(q, k, v, S, Wq, Wk, w):
    q = np.asarray(q, dtype=np.float32)
    k = np.asarray(k, dtype=np.float32)
    S_np = np.asarray(S).astype(np.int64)
    Wq = np.asarray(Wq, dtype=np.float32)
    Wk = np.asarray(Wk, dtype=np.float32)
    w = np.asarray(w, dtype=np.float32)

    nb = tuple(int(min(K, max(32, _round_up(int(s), 32)))) for s in S_np)

    if nb not in _CACHE:
        _CACHE[nb] = build(nb)
    nc = _CACHE[nb]

    wqT = np.ascontiguousarray(Wq.T)  # (D, H)
    wkT = np.ascontiguousarray(Wk.T)  # (D, H)
    w_r = np.ascontiguousarray(w.reshape(2, 128, 1))

    in_maps = []
    for c in range(NCORES):
        in_maps.append(
            {
                "q": np.ascontiguousarray(q[:, c * QSH : (c + 1) * QSH, :]),
                "k": k,
                "wqT": wqT,
                "wkT": wkT,
                "w": w_r,
            }
        )

    res = run_bass_kernel_spmd(nc, in_maps, core_ids=list(range(NCORES)))
    outs = res.results

    full = np.empty((B, Q, K), dtype=np.float32)
    for c in range(NCORES):
        full[:, c * QSH : (c + 1) * QSH, :] = outs[c]["out"]
    for b in range(B):
        full[b, :, int(S_np[b]) :] = -np.inf
    return full
